# revision 29
# baseline (speedup 1.0000x reference)
"""KroneckerLinear Trainium2 kernel (v2 — bf16 + DVE stream-transpose).

y[b,t,o*64+p] = sum_{s,i,j} A[s,o,i] * x[b,t,i*64+j] * B[s,p,j] + bias[o*64+p]

Data-parallel over the 16384 tokens, 2048 per core; per token the op is
Y_t = sum_s A_s @ X_t @ B_s^T with X_t = x_t.reshape(64,64).

On-chip dataflow per 16-token tile (two 8-token half-groups h=0,1):
  MM1 (PE, 4 quadrant-concurrent 64x64 matmuls):
      V[(p1,s,p0), (h,r,i1,i0)] = sum_j btP[(tau,j),(p1,s,p0)] * X[(tau,j),(r,i)]
    with partition index (p1,s,p0) = p1*64+s*32+p0 (p = p1*32+p0) chosen so the
    Kronecker swap i<->p becomes a pure 32x32-block-local transpose.
  SWAP (DVE, ONE InstStreamTranspose per tile): fp32 PSUM -> bf16 SBUF,
      G[(p1,s,i0), (h,r,i1,p0)] = V[(p1,s,p0), (h,r,i1,i0)]
    (each 32x32 block transposed in place; the (p1,s) partition-block and the
    (h,r,i1) free-chunk indices are preserved — exactly the layout MM2 needs).
  MM2 (PE, 8 small matmuls, PSUM-accumulated over i1):
      Y[(h,o), (r,p1,p0)] += sum_{(s,i0)} A2_i1[(p1,s,i0),(h,o)] * G[...]
  Y evac (ACT): fp32 PSUM -> bf16 SBUF; bias is added on the host.

Everything that crosses HBM is bf16 (x in, y out) — host converts for free.
"""

import numpy as np
from ml_dtypes import bfloat16

IN1 = IN2 = OUT1 = OUT2 = 64
NUM_SUM = 2
BATCH, SEQ = 4, 4096
NCORES = 8
TOK = BATCH * SEQ            # 16384 tokens
TPC = TOK // NCORES          # 2048 tokens per core
TILE_TOK = 16                # tokens per on-chip tile (two 8-token halves)
NT = TPC // TILE_TOK         # 128 tiles per core

_cached = {}


def _build_bass(nt=NT):
    import os
    import concourse.bass as bass
    import concourse.mybir as mybir
    from concourse import bacc, tile

    ys_f32 = bool(int(os.environ.get("KV_YS_F32", "0")))
    no_st = bool(int(os.environ.get("KV_NOST", "0")))
    no_mm2 = bool(int(os.environ.get("KV_NOMM2", "0")))

    f32 = mybir.dt.float32
    bf16 = mybir.dt.bfloat16
    ydt = f32 if ys_f32 else bf16
    nc = bacc.Bacc(None, target_bir_lowering=False, debug=False)

    xdev = nc.declare_dram_parameter("xdev", [128, nt * 512], bf16, isOutput=False)
    btp_d = nc.declare_dram_parameter("btp", [128, 128], bf16, isOutput=False)
    a20_d = nc.declare_dram_parameter("a20", [128, 128], f32, isOutput=False)
    a21_d = nc.declare_dram_parameter("a21", [128, 128], f32, isOutput=False)
    ydev = nc.declare_dram_parameter("ydev", [128, nt * 512], ydt, isOutput=True)

    with tile.TileContext(nc) as tc:
        with (
            tc.tile_pool(name="consts", bufs=1) as cpool,
            tc.tile_pool(name="xs", bufs=4) as xpool,
            tc.tile_pool(name="gsb", bufs=3) as gpool,
            tc.tile_pool(name="ysb", bufs=4) as ypool,
            tc.tile_pool(name="vps", bufs=4, space="PSUM") as vpsum,
            tc.tile_pool(name="yps", bufs=2, space="PSUM") as ypsum,
        ):
            btp = cpool.tile([128, 128], bf16)
            a2 = [cpool.tile([128, 128], f32, tag=f"a2_{i1}", name=f"a2_{i1}")
                  for i1 in range(2)]
            nc.sync.dma_start(out=btp, in_=btp_d[:, :])
            nc.sync.dma_start(out=a2[0], in_=a20_d[:, :])
            nc.sync.dma_start(out=a2[1], in_=a21_d[:, :])

            per = 2 if nt % 2 == 0 else 1
            for gp in range(nt // per):
                # One input DMA per tile-pair (2KB/partition) on the sync
                # HWDGE queue; the output DMA per pair goes to the gpsimd
                # SWDGE queue so the two streams don't serialize on the SP
                # sequencer (565ns setup per HWDGE dma).
                xs = xpool.tile([128, 2, 512], bf16, tag="xs")
                nc.sync.dma_start(
                    out=xs[:, 0:per, :],
                    in_=xdev[:, gp * per * 512:(gp * per + per) * 512])
                ys = ypool.tile([128, 2, 512], ydt, tag="ys")

                for t in range(per):
                    # One V half-tile (one PSUM bank) per token-half h; the
                    # swap and MM2 for each half proceed independently, so
                    # the PE never waits long on the DVE (and HAM stays warm).
                    yp = None if no_mm2 else ypsum.tile([128, 2, 512], f32, tag="y")
                    for h in range(2):
                        # MM1: 2 concurrent 64x64-quadrant matmuls (p1 cols).
                        # x free layout is (i1, r, i0) so every later operand
                        # slice is a contiguous 1-D run (HW rejects multi-dim
                        # matmul APs).
                        vp = vpsum.tile([128, 512], f32, tag="v",
                                        name=f"vp{h}")
                        for p1 in range(2):
                            nc.tensor.matmul(
                                vp[p1 * 64:(p1 + 1) * 64, :],
                                lhsT=btp[h * 64:(h + 1) * 64, p1 * 64:(p1 + 1) * 64],
                                rhs=xs[h * 64:(h + 1) * 64, t, :],
                                start=True, stop=True,
                                tile_position=(h * 64, p1 * 64),
                            )

                        # Kronecker swap: DVE stream-transpose (32x32 blocks),
                        # PSUM -> SBUF, same dtype (fp32) per the ISA rule.
                        gs = gpool.tile([128, 2, 256], f32, tag="gs",
                                        name=f"gs{h}")
                        if no_st:
                            nc.vector.tensor_copy(gs[:, :, :], vp[:, :])
                        else:
                            nc.vector.transpose(gs[:, :, :], vp[:, :])

                        # MM2: Y[(h,o),(p1,r,p0)] = sum_{i1,(s,i0)} A2 * G,
                        # accumulated over i1 in PSUM; quadrants (p1 rows, h
                        # cols). PSUM matmul dst must start at a bank
                        # boundary: each p1 gets its own bank (half-used).
                        if no_mm2:
                            if h == 0:
                                nc.scalar.copy(ys[:, t, :], gs[:, :, :])
                        else:
                            for p1 in range(2):
                                for i1 in range(2):
                                    nc.tensor.matmul(
                                        yp[h * 64:(h + 1) * 64, p1, 0:256],
                                        lhsT=a2[i1][p1 * 64:(p1 + 1) * 64, h * 64:(h + 1) * 64],
                                        rhs=gs[p1 * 64:(p1 + 1) * 64, i1, :],
                                        start=(i1 == 0), stop=(i1 == 1),
                                        tile_position=(p1 * 64, h * 64),
                                    )
                    if not no_mm2:
                        nc.scalar.copy(ys[:, t, :], yp[:, :, 0:256])
                nc.gpsimd.dma_start(
                    out=ydev[:, gp * per * 512:(gp * per + per) * 512],
                    in_=ys[:, 0:per, :])

    nc.finalize()
    return nc


def _get_nc():
    if "nc" not in _cached:
        _cached["nc"] = _build_bass()
    return _cached["nc"]


def _host_prep_x(xc, nt=NT):
    # xc: (nt*16, 4096) bf16 tokens for one core ->
    # xdev[tau*64+j, g*512 + i1*256 + r*32 + i0] =
    #     xc[g*16 + tau*8 + r, (i1*32+i0)*64 + j]
    x4 = xc.reshape(nt, 2, 8, 2, 32, IN2)         # g, tau, r, i1, i0, j
    xd = x4.transpose(1, 5, 0, 3, 2, 4)           # tau, j, g, i1, r, i0
    return np.ascontiguousarray(xd).reshape(128, nt * 512)


def _host_post_y(yd, nt=NT):
    # yd: (128, nt*512) bf16;
    # yd[h*64+o, g*512 + p1*256 + r*32 + p0] = yc[g*16+h*8+r, o*64 + p1*32 + p0]
    y6 = yd.reshape(2, OUT1, nt, 2, 8, 32)        # h, o, g, p1, r, p0
    yc = y6.transpose(2, 0, 4, 1, 3, 5)           # g, h, r, o, p1, p0
    return np.ascontiguousarray(yc).reshape(nt * TILE_TOK, OUT1 * OUT2)


def _make_in_maps(x, A, B, bias):
    A = np.asarray(A, np.float32)
    B = np.asarray(B, np.float32)
    xf = np.asarray(x, np.float32).reshape(TOK, IN1 * IN2).astype(bfloat16)

    # btp[tau*64+j, p1*64+s*32+p0] = B[s, p1*32+p0, j]
    b4 = B.reshape(NUM_SUM, 2, 32, IN2)           # s, p1, p0, j
    bt = b4.transpose(3, 1, 0, 2).reshape(IN2, 128)   # j, (p1,s,p0)
    btp = np.ascontiguousarray(np.concatenate([bt, bt], 0)).astype(bfloat16)

    # a2[i1][p1*64+s*32+i0, h*64+o] = A[s, o, i1*32+i0]
    a4 = A.reshape(NUM_SUM, OUT1, 2, 32)          # s, o, i1, i0
    a2 = []
    for i1 in range(2):
        blk = a4[:, :, i1, :].transpose(0, 2, 1).reshape(64, OUT1)  # (s,i0), o
        full = np.concatenate([blk, blk], 0)       # p1 duplication -> (128, 64)
        full = np.concatenate([full, full], 1)     # h duplication  -> (128, 128)
        a2.append(np.ascontiguousarray(full, np.float32))

    in_maps = []
    for c in range(NCORES):
        xc = xf[c * TPC:(c + 1) * TPC]
        in_maps.append({
            "xdev": _host_prep_x(xc),
            "btp": btp,
            "a20": a2[0],
            "a21": a2[1],
        })
    return in_maps


def _run(inputs, trace=False, **kw):
    from concourse.bass_utils import run_bass_kernel_spmd

    nc = _get_nc()
    in_maps = _make_in_maps(**inputs)
    res = run_bass_kernel_spmd(nc, in_maps, core_ids=list(range(NCORES)),
                               trace=trace, **kw)
    shards = [_host_post_y(np.asarray(res.results[c]["ydev"]))
              for c in range(NCORES)]
    y = np.concatenate(shards, 0).reshape(BATCH, SEQ, OUT1 * OUT2)
    y = y.astype(np.float32) + np.asarray(inputs["bias"], np.float32)
    return y, res


def kernel(x, A, B, bias):
    y, _ = _run(dict(x=x, A=A, B=B, bias=bias), trace=False)
    return y


# revision 30
# speedup vs baseline: 1.4884x; 1.4884x over previous
"""KroneckerLinear Trainium2 kernel (v2 — bf16 + DVE stream-transpose).

y[b,t,o*64+p] = sum_{s,i,j} A[s,o,i] * x[b,t,i*64+j] * B[s,p,j] + bias[o*64+p]

Data-parallel over the 16384 tokens, 2048 per core; per token the op is
Y_t = sum_s A_s @ X_t @ B_s^T with X_t = x_t.reshape(64,64).

On-chip dataflow per 16-token tile (two 8-token half-groups h=0,1):
  MM1 (PE, 4 quadrant-concurrent 64x64 matmuls):
      V[(p1,s,p0), (h,r,i1,i0)] = sum_j btP[(tau,j),(p1,s,p0)] * X[(tau,j),(r,i)]
    with partition index (p1,s,p0) = p1*64+s*32+p0 (p = p1*32+p0) chosen so the
    Kronecker swap i<->p becomes a pure 32x32-block-local transpose.
  SWAP (DVE, ONE InstStreamTranspose per tile): fp32 PSUM -> bf16 SBUF,
      G[(p1,s,i0), (h,r,i1,p0)] = V[(p1,s,p0), (h,r,i1,i0)]
    (each 32x32 block transposed in place; the (p1,s) partition-block and the
    (h,r,i1) free-chunk indices are preserved — exactly the layout MM2 needs).
  MM2 (PE, 8 small matmuls, PSUM-accumulated over i1):
      Y[(h,o), (r,p1,p0)] += sum_{(s,i0)} A2_i1[(p1,s,i0),(h,o)] * G[...]
  Y evac (ACT): fp32 PSUM -> bf16 SBUF; bias is added on the host.

Everything that crosses HBM is bf16 (x in, y out) — host converts for free.
"""

import numpy as np
from ml_dtypes import bfloat16

IN1 = IN2 = OUT1 = OUT2 = 64
NUM_SUM = 2
BATCH, SEQ = 4, 4096
NCORES = 8
TOK = BATCH * SEQ            # 16384 tokens
TPC = TOK // NCORES          # 2048 tokens per core
TILE_TOK = 16                # tokens per on-chip tile (two 8-token halves)
NT = TPC // TILE_TOK         # 128 tiles per core

_cached = {}


def _build_bass(nt=NT):
    import os
    import concourse.bass as bass
    import concourse.mybir as mybir
    from concourse import bacc, tile

    ys_f32 = bool(int(os.environ.get("KV_YS_F32", "0")))
    no_st = bool(int(os.environ.get("KV_NOST", "0")))
    no_mm2 = bool(int(os.environ.get("KV_NOMM2", "0")))

    f32 = mybir.dt.float32
    bf16 = mybir.dt.bfloat16
    ydt = f32 if ys_f32 else bf16
    nc = bacc.Bacc(None, target_bir_lowering=False, debug=False)

    xdev = nc.declare_dram_parameter("xdev", [128, nt * 512], bf16, isOutput=False)
    btp_d = nc.declare_dram_parameter("btp", [128, 128], bf16, isOutput=False)
    a20_d = nc.declare_dram_parameter("a20", [128, 128], f32, isOutput=False)
    a21_d = nc.declare_dram_parameter("a21", [128, 128], f32, isOutput=False)
    ydev = nc.declare_dram_parameter("ydev", [128, nt * 512], ydt, isOutput=True)

    with tile.TileContext(nc) as tc:
        with (
            tc.tile_pool(name="consts", bufs=1) as cpool,
            tc.tile_pool(name="xs", bufs=4) as xpool,
            tc.tile_pool(name="gsb", bufs=4) as gpool,
            tc.tile_pool(name="ysb", bufs=4) as ypool,
            tc.tile_pool(name="vps", bufs=2, space="PSUM") as vpsum,
            tc.tile_pool(name="yps", bufs=2, space="PSUM") as ypsum,
        ):
            btp = cpool.tile([128, 128], bf16)
            a2 = [cpool.tile([128, 128], f32, tag=f"a2_{i1}", name=f"a2_{i1}")
                  for i1 in range(2)]
            nc.sync.dma_start(out=btp, in_=btp_d[:, :])
            nc.sync.dma_start(out=a2[0], in_=a20_d[:, :])
            nc.sync.dma_start(out=a2[1], in_=a21_d[:, :])

            per = 2 if nt % 2 == 0 else 1
            L = 2 if nt > 2 else 0   # software-pipeline lookahead (tiles)
            xs_t, gs_t, ys_t = {}, {}, {}

            def emit_front(t):
                # DMA-in per pair (2KB/partition, sync HWDGE queue), MM1,
                # and the DVE Kronecker swap for tile t.
                if t % per == 0:
                    xs = xpool.tile([128, 2, 512], bf16, tag="xs", name="xs")
                    xs_t[t // per] = xs
                    nc.sync.dma_start(
                        out=xs[:, 0:per, :],
                        in_=xdev[:, t * 512:(t + per) * 512])
                xs = xs_t[t // per]
                # MM1: 4 concurrent 64x64-quadrant matmuls (h rows, p1 cols).
                # x free layout is (i1, r, i0) so every operand slice is a
                # contiguous 1-D run (HW rejects multi-dim matmul APs).
                vp = vpsum.tile([128, 2, 512], f32, tag="v", name="vp")
                for h in range(2):
                    for p1 in range(2):
                        nc.tensor.matmul(
                            vp[p1 * 64:(p1 + 1) * 64, h, :],
                            lhsT=btp[h * 64:(h + 1) * 64, p1 * 64:(p1 + 1) * 64],
                            rhs=xs[h * 64:(h + 1) * 64, t % per, :],
                            start=True, stop=True,
                            tile_position=(h * 64, p1 * 64),
                        )
                # Kronecker swap: one DVE stream-transpose (32x32 blocks),
                # PSUM -> SBUF, same dtype (fp32) per the ISA rule.
                gs = gpool.tile([128, 2, 2, 256], f32, tag="gs", name="gs")
                gs_t[t] = gs
                nc.vector.transpose(gs[:, :, :, :], vp[:, :, :])

            def emit_back(t):
                # MM2 + Y evacuation for tile t (emitted L tiles later so the
                # PE never stalls on the DVE swap), DMA-out per pair (SWDGE).
                gs = gs_t.pop(t)
                if t % per == 0:
                    ys_t[t // per] = ypool.tile([128, 2, 512], ydt,
                                                tag="ys", name="ys")
                ys = ys_t[t // per]
                # MM2: Y[(h,o),(p1,r,p0)] = sum_{i1,(s,i0)} A2 * G, PSUM-
                # accumulated over i1; 4 quadrant positions (p1 rows, h
                # cols). PSUM matmul dst must start at a bank boundary:
                # each p1 gets its own bank (half-used), gathered at evac.
                yp = ypsum.tile([128, 2, 512], f32, tag="y", name="yp")
                for h in range(2):
                    for p1 in range(2):
                        for i1 in range(2):
                            nc.tensor.matmul(
                                yp[h * 64:(h + 1) * 64, p1, 0:256],
                                lhsT=a2[i1][p1 * 64:(p1 + 1) * 64, h * 64:(h + 1) * 64],
                                rhs=gs[p1 * 64:(p1 + 1) * 64, h, i1, :],
                                start=(i1 == 0), stop=(i1 == 1),
                                tile_position=(p1 * 64, h * 64),
                            )
                nc.scalar.copy(ys[:, t % per, :], yp[:, :, 0:256])
                if t % per == per - 1:
                    nc.gpsimd.dma_start(
                        out=ydev[:, (t - per + 1) * 512:(t + 1) * 512],
                        in_=ys_t.pop(t // per)[:, 0:per, :])

            for t in range(nt):
                emit_front(t)
                if t >= L:
                    emit_back(t - L)
            for t in range(max(0, nt - L), nt):
                emit_back(t)

    nc.finalize()
    return nc


def _get_nc():
    if "nc" not in _cached:
        _cached["nc"] = _build_bass()
    return _cached["nc"]


def _host_prep_x(xc, nt=NT):
    # xc: (nt*16, 4096) bf16 tokens for one core ->
    # xdev[tau*64+j, g*512 + i1*256 + r*32 + i0] =
    #     xc[g*16 + tau*8 + r, (i1*32+i0)*64 + j]
    x4 = xc.reshape(nt, 2, 8, 2, 32, IN2)         # g, tau, r, i1, i0, j
    xd = x4.transpose(1, 5, 0, 3, 2, 4)           # tau, j, g, i1, r, i0
    return np.ascontiguousarray(xd).reshape(128, nt * 512)


def _host_post_y(yd, nt=NT):
    # yd: (128, nt*512) bf16;
    # yd[h*64+o, g*512 + p1*256 + r*32 + p0] = yc[g*16+h*8+r, o*64 + p1*32 + p0]
    y6 = yd.reshape(2, OUT1, nt, 2, 8, 32)        # h, o, g, p1, r, p0
    yc = y6.transpose(2, 0, 4, 1, 3, 5)           # g, h, r, o, p1, p0
    return np.ascontiguousarray(yc).reshape(nt * TILE_TOK, OUT1 * OUT2)


def _make_in_maps(x, A, B, bias):
    A = np.asarray(A, np.float32)
    B = np.asarray(B, np.float32)
    xf = np.asarray(x, np.float32).reshape(TOK, IN1 * IN2).astype(bfloat16)

    # btp[tau*64+j, p1*64+s*32+p0] = B[s, p1*32+p0, j]
    b4 = B.reshape(NUM_SUM, 2, 32, IN2)           # s, p1, p0, j
    bt = b4.transpose(3, 1, 0, 2).reshape(IN2, 128)   # j, (p1,s,p0)
    btp = np.ascontiguousarray(np.concatenate([bt, bt], 0)).astype(bfloat16)

    # a2[i1][p1*64+s*32+i0, h*64+o] = A[s, o, i1*32+i0]
    a4 = A.reshape(NUM_SUM, OUT1, 2, 32)          # s, o, i1, i0
    a2 = []
    for i1 in range(2):
        blk = a4[:, :, i1, :].transpose(0, 2, 1).reshape(64, OUT1)  # (s,i0), o
        full = np.concatenate([blk, blk], 0)       # p1 duplication -> (128, 64)
        full = np.concatenate([full, full], 1)     # h duplication  -> (128, 128)
        a2.append(np.ascontiguousarray(full, np.float32))

    in_maps = []
    for c in range(NCORES):
        xc = xf[c * TPC:(c + 1) * TPC]
        in_maps.append({
            "xdev": _host_prep_x(xc),
            "btp": btp,
            "a20": a2[0],
            "a21": a2[1],
        })
    return in_maps


def _run(inputs, trace=False, **kw):
    from concourse.bass_utils import run_bass_kernel_spmd

    nc = _get_nc()
    in_maps = _make_in_maps(**inputs)
    res = run_bass_kernel_spmd(nc, in_maps, core_ids=list(range(NCORES)),
                               trace=trace, **kw)
    shards = [_host_post_y(np.asarray(res.results[c]["ydev"]))
              for c in range(NCORES)]
    y = np.concatenate(shards, 0).reshape(BATCH, SEQ, OUT1 * OUT2)
    y = y.astype(np.float32) + np.asarray(inputs["bias"], np.float32)
    return y, res


def kernel(x, A, B, bias):
    y, _ = _run(dict(x=x, A=A, B=B, bias=bias), trace=False)
    return y


# revision 32
# speedup vs baseline: 1.7746x; 1.1923x over previous
"""KroneckerLinear Trainium2 kernel (v2 — bf16 + DVE stream-transpose).

y[b,t,o*64+p] = sum_{s,i,j} A[s,o,i] * x[b,t,i*64+j] * B[s,p,j] + bias[o*64+p]

Data-parallel over the 16384 tokens, 2048 per core; per token the op is
Y_t = sum_s A_s @ X_t @ B_s^T with X_t = x_t.reshape(64,64).

On-chip dataflow per 16-token tile (two 8-token half-groups h=0,1):
  MM1 (PE, 4 quadrant-concurrent 64x64 matmuls):
      V[(p1,s,p0), (h,r,i1,i0)] = sum_j btP[(tau,j),(p1,s,p0)] * X[(tau,j),(r,i)]
    with partition index (p1,s,p0) = p1*64+s*32+p0 (p = p1*32+p0) chosen so the
    Kronecker swap i<->p becomes a pure 32x32-block-local transpose.
  SWAP (DVE, ONE InstStreamTranspose per tile): fp32 PSUM -> bf16 SBUF,
      G[(p1,s,i0), (h,r,i1,p0)] = V[(p1,s,p0), (h,r,i1,i0)]
    (each 32x32 block transposed in place; the (p1,s) partition-block and the
    (h,r,i1) free-chunk indices are preserved — exactly the layout MM2 needs).
  MM2 (PE, 8 small matmuls, PSUM-accumulated over i1):
      Y[(h,o), (r,p1,p0)] += sum_{(s,i0)} A2_i1[(p1,s,i0),(h,o)] * G[...]
  Y evac (ACT): fp32 PSUM -> bf16 SBUF; bias is added on the host.

Everything that crosses HBM is bf16 (x in, y out) — host converts for free.
"""

import numpy as np
from ml_dtypes import bfloat16

IN1 = IN2 = OUT1 = OUT2 = 64
NUM_SUM = 2
BATCH, SEQ = 4, 4096
NCORES = 8
TOK = BATCH * SEQ            # 16384 tokens
TPC = TOK // NCORES          # 2048 tokens per core
TILE_TOK = 16                # tokens per on-chip tile (two 8-token halves)
NT = TPC // TILE_TOK         # 128 tiles per core

_cached = {}


def _build_bass(nt=NT):
    import os
    import concourse.bass as bass
    import concourse.mybir as mybir
    from concourse import bacc, tile

    ys_f32 = bool(int(os.environ.get("KV_YS_F32", "0")))
    no_st = bool(int(os.environ.get("KV_NOST", "0")))
    no_mm2 = bool(int(os.environ.get("KV_NOMM2", "0")))

    f32 = mybir.dt.float32
    bf16 = mybir.dt.bfloat16
    ydt = f32 if ys_f32 else bf16
    nc = bacc.Bacc(None, target_bir_lowering=False, debug=False)

    xdev = nc.declare_dram_parameter("xdev", [128, nt * 512], bf16, isOutput=False)
    btp_d = nc.declare_dram_parameter("btp", [128, 128], bf16, isOutput=False)
    a20_d = nc.declare_dram_parameter("a20", [128, 128], f32, isOutput=False)
    a21_d = nc.declare_dram_parameter("a21", [128, 128], f32, isOutput=False)
    ydev = nc.declare_dram_parameter("ydev", [128, nt * 512], ydt, isOutput=True)

    with tile.TileContext(nc) as tc:
        with (
            tc.tile_pool(name="consts", bufs=1) as cpool,
            tc.tile_pool(name="xs", bufs=4) as xpool,
            tc.tile_pool(name="gsb", bufs=5) as gpool,
            tc.tile_pool(name="ysb", bufs=4) as ypool,
            tc.tile_pool(name="vps", bufs=2, space="PSUM") as vpsum,
            tc.tile_pool(name="yps", bufs=2, space="PSUM") as ypsum,
        ):
            btp = cpool.tile([128, 128], bf16)
            a2 = [cpool.tile([128, 128], f32, tag=f"a2_{i1}", name=f"a2_{i1}")
                  for i1 in range(2)]
            nc.sync.dma_start(out=btp, in_=btp_d[:, :])
            nc.sync.dma_start(out=a2[0], in_=a20_d[:, :])
            nc.sync.dma_start(out=a2[1], in_=a21_d[:, :])

            per = 2 if nt % 2 == 0 else 1
            L = 3 if nt > 3 else 0   # software-pipeline lookahead (tiles)
            xs_t, gs_t, ys_t = {}, {}, {}

            def emit_front(t):
                # DMA-in per pair (2KB/partition, sync HWDGE queue), MM1,
                # and the DVE Kronecker swap for tile t.
                if t % per == 0:
                    xs = xpool.tile([128, 2, 512], bf16, tag="xs", name="xs")
                    xs_t[t // per] = xs
                    nc.sync.dma_start(
                        out=xs[:, 0:per, :],
                        in_=xdev[:, t * 512:(t + per) * 512])
                xs = xs_t[t // per]
                # MM1: 4 concurrent 64x64-quadrant matmuls (h rows, p1 cols).
                # x free layout is (i1, r, i0) so every operand slice is a
                # contiguous 1-D run (HW rejects multi-dim matmul APs).
                vp = vpsum.tile([128, 2, 512], f32, tag="v", name="vp")
                for h in range(2):
                    for p1 in range(2):
                        nc.tensor.matmul(
                            vp[p1 * 64:(p1 + 1) * 64, h, :],
                            lhsT=btp[h * 64:(h + 1) * 64, p1 * 64:(p1 + 1) * 64],
                            rhs=xs[h * 64:(h + 1) * 64, t % per, :],
                            start=True, stop=True,
                            tile_position=(h * 64, p1 * 64),
                        )
                # Kronecker swap: one DVE stream-transpose (32x32 blocks),
                # PSUM -> SBUF, same dtype (fp32) per the ISA rule.
                gs = gpool.tile([128, 2, 2, 256], f32, tag="gs", name="gs")
                gs_t[t] = gs
                nc.vector.transpose(gs[:, :, :, :], vp[:, :, :])

            def emit_back(t):
                # MM2 + Y evacuation for tile t (emitted L tiles later so the
                # PE never stalls on the DVE swap), DMA-out per pair (SWDGE).
                gs = gs_t.pop(t)
                if t % per == 0:
                    ys_t[t // per] = ypool.tile([128, 2, 512], ydt,
                                                tag="ys", name="ys")
                ys = ys_t[t // per]
                # MM2: Y[(h,o),(p1,r,p0)] = sum_{i1,(s,i0)} A2 * G, PSUM-
                # accumulated over i1; 4 quadrant positions (p1 rows, h
                # cols). PSUM matmul dst must start at a bank boundary:
                # each p1 gets its own bank (half-used), gathered at evac.
                yp = ypsum.tile([128, 2, 512], f32, tag="y", name="yp")
                for h in range(2):
                    for p1 in range(2):
                        for i1 in range(2):
                            nc.tensor.matmul(
                                yp[h * 64:(h + 1) * 64, p1, 0:256],
                                lhsT=a2[i1][p1 * 64:(p1 + 1) * 64, h * 64:(h + 1) * 64],
                                rhs=gs[p1 * 64:(p1 + 1) * 64, h, i1, :],
                                start=(i1 == 0), stop=(i1 == 1),
                                tile_position=(p1 * 64, h * 64),
                            )
                nc.scalar.copy(ys[:, t % per, :], yp[:, :, 0:256])
                if t % per == per - 1:
                    nc.gpsimd.dma_start(
                        out=ydev[:, (t - per + 1) * 512:(t + 1) * 512],
                        in_=ys_t.pop(t // per)[:, 0:per, :])

            for t in range(nt):
                emit_front(t)
                if t >= L:
                    emit_back(t - L)
            for t in range(max(0, nt - L), nt):
                emit_back(t)

    nc.finalize()
    return nc


def _get_nc():
    if "nc" not in _cached:
        _cached["nc"] = _build_bass()
    return _cached["nc"]


def _host_prep_x(xc, nt=NT):
    # xc: (nt*16, 4096) bf16 tokens for one core ->
    # xdev[tau*64+j, g*512 + i1*256 + r*32 + i0] =
    #     xc[g*16 + tau*8 + r, (i1*32+i0)*64 + j]
    x4 = xc.reshape(nt, 2, 8, 2, 32, IN2)         # g, tau, r, i1, i0, j
    xd = x4.transpose(1, 5, 0, 3, 2, 4)           # tau, j, g, i1, r, i0
    return np.ascontiguousarray(xd).reshape(128, nt * 512)


def _host_post_y(yd, nt=NT):
    # yd: (128, nt*512) bf16;
    # yd[h*64+o, g*512 + p1*256 + r*32 + p0] = yc[g*16+h*8+r, o*64 + p1*32 + p0]
    y6 = yd.reshape(2, OUT1, nt, 2, 8, 32)        # h, o, g, p1, r, p0
    yc = y6.transpose(2, 0, 4, 1, 3, 5)           # g, h, r, o, p1, p0
    return np.ascontiguousarray(yc).reshape(nt * TILE_TOK, OUT1 * OUT2)


def _make_in_maps(x, A, B, bias):
    A = np.asarray(A, np.float32)
    B = np.asarray(B, np.float32)
    xf = np.asarray(x, np.float32).reshape(TOK, IN1 * IN2).astype(bfloat16)

    # btp[tau*64+j, p1*64+s*32+p0] = B[s, p1*32+p0, j]
    b4 = B.reshape(NUM_SUM, 2, 32, IN2)           # s, p1, p0, j
    bt = b4.transpose(3, 1, 0, 2).reshape(IN2, 128)   # j, (p1,s,p0)
    btp = np.ascontiguousarray(np.concatenate([bt, bt], 0)).astype(bfloat16)

    # a2[i1][p1*64+s*32+i0, h*64+o] = A[s, o, i1*32+i0]
    a4 = A.reshape(NUM_SUM, OUT1, 2, 32)          # s, o, i1, i0
    a2 = []
    for i1 in range(2):
        blk = a4[:, :, i1, :].transpose(0, 2, 1).reshape(64, OUT1)  # (s,i0), o
        full = np.concatenate([blk, blk], 0)       # p1 duplication -> (128, 64)
        full = np.concatenate([full, full], 1)     # h duplication  -> (128, 128)
        a2.append(np.ascontiguousarray(full, np.float32))

    in_maps = []
    for c in range(NCORES):
        xc = xf[c * TPC:(c + 1) * TPC]
        in_maps.append({
            "xdev": _host_prep_x(xc),
            "btp": btp,
            "a20": a2[0],
            "a21": a2[1],
        })
    return in_maps


def _run(inputs, trace=False, **kw):
    from concourse.bass_utils import run_bass_kernel_spmd

    nc = _get_nc()
    in_maps = _make_in_maps(**inputs)
    res = run_bass_kernel_spmd(nc, in_maps, core_ids=list(range(NCORES)),
                               trace=trace, **kw)
    shards = [_host_post_y(np.asarray(res.results[c]["ydev"]))
              for c in range(NCORES)]
    y = np.concatenate(shards, 0).reshape(BATCH, SEQ, OUT1 * OUT2)
    y = y.astype(np.float32) + np.asarray(inputs["bias"], np.float32)
    return y, res


def kernel(x, A, B, bias):
    y, _ = _run(dict(x=x, A=A, B=B, bias=bias), trace=False)
    return y
